# revision 24
# baseline (speedup 1.0000x reference)
"""Distributed Trainium2 kernel for nn_Attention_25228637897408.

GQA attention (B=1, T=2048, D=2048, NH=16, NKV=4, HD=128) with RoPE,
per-head rms_norm, skip-gate blend of k/v, v_bias, causal softmax and
output projection, tensor-parallel over heads on 8 NeuronCores.

Per-core work (core c):
  - q-heads {2c, 2c+1}, kv-head c//2.
  - skip blend done ONCE on activations: stb = x + (g/(1-g))*skip (DVE
    scalar_tensor_tensor); k/v projections contract stb against
    (1-g)-scaled weights -> halves the k/v matmul count.
  - phase 1 streams x/skip in 1024-token halves; each projection unit
    is a dense 32-MM burst whose raw result is copied straight into the
    persistent kT/qT tiles; rms_norm+RoPE epilogues are batched per
    1024-token row and interleaved into later MM bursts so the PE never
    starves (keeps the HAM clock warm).
  - partition reductions (sum-of-squares, softmax denominator) are
    single ones-column matmuls into [1,512] PSUM rows; 1/x and
    1/sqrt(x) via exp(-ln(x)) on ACT: the whole kernel uses one
    activation table set (natural_log_exp_and_others).
  - attention per 128-k-tile: scores -> exp -> y/l accumulate, emitted
    software-pipelined (scores(kt+1) ahead of y/l(kt)) so exp latency
    hides under PE work; causal mask added on the PE via identity x
    mask matmul; exp with per-head scale=gain^2/sqrt(HD) and
    bias=-gain^2*sqrt(HD).
  - AllToAll (gpsimd stays free of DMA work so triggers fire promptly)
    redistributes y; the output projection is split into an h0-block
    pass that overlaps A2A(h1)'s flight and an h1-block pass (8 PSUM
    banks held across the passes).
Host side only reshapes/transposes/casts and slices inputs; all value
computation (sigmoid, blending, norms, softmax, matmuls) is on device.
"""

import sys

sys.path.insert(0, "/opt/trn_rl_repo")

import numpy as np
import ml_dtypes

import concourse.bass as bass
import concourse.mybir as mybir
import concourse.tile as tile
from concourse import bacc
from concourse.bass_utils import run_bass_kernel_spmd

BF16 = ml_dtypes.bfloat16

T = 2048
D = 2048
NH = 16
NKV = 4
HD = 128
REP = NH // NKV
NCORES = 8
HQ = NH // NCORES  # q heads per core = 2
ROPE_BASE = 10000.0
EPS = float(np.finfo(np.float32).eps)
MASK_VAL = -1.0e5

dt = mybir.dt
AF = mybir.ActivationFunctionType
ALU = mybir.AluOpType


def _bf(x):
    return np.ascontiguousarray(np.asarray(x, dtype=np.float32)).astype(BF16)


def build_graph(t=T):
    """Build the SPMD graph (identical on all cores). t parametrizes the
    sequence length for simulator-sized testing."""
    assert t % 1024 == 0
    n_ch = t // 512  # 512-wide token chunks
    n_kt = t // 128  # 128-row tiles along T
    kpc = n_kt // n_ch  # k-tiles per chunk = 4
    rows = t // NCORES  # output rows per core
    n_dt = D // 128  # tiles along D contraction = 16
    HW = 1024  # phase-1 half width
    n_half = t // HW

    nc = bacc.Bacc(None, target_bir_lowering=False)

    xT_d = nc.declare_dram_parameter("xT", [D, t], dt.bfloat16, isOutput=False)
    skT_d = nc.declare_dram_parameter("skipT", [D, t], dt.bfloat16, isOutput=False)
    wqT_d = nc.declare_dram_parameter("wqT", [D, HQ * HD], dt.bfloat16, isOutput=False)
    wkT_d = nc.declare_dram_parameter("wkT", [D, HD], dt.bfloat16, isOutput=False)
    wvT_d = nc.declare_dram_parameter("wvT", [D, HD], dt.bfloat16, isOutput=False)
    wpT_d = nc.declare_dram_parameter("wprojT", [D, D], dt.bfloat16, isOutput=False)
    qkg_d = nc.declare_dram_parameter("qkg", [1, HQ], dt.float32, isOutput=False)
    lns_d = nc.declare_dram_parameter("lns", [1, 1], dt.float32, isOutput=False)
    vb_d = nc.declare_dram_parameter("vbias", [1, HD], dt.float32, isOutput=False)
    cosF_d = nc.declare_dram_parameter("cosF", [HD, t], dt.bfloat16, isOutput=False)
    sinF_d = nc.declare_dram_parameter("sinF", [HD, t], dt.bfloat16, isOutput=False)
    mask_d = nc.declare_dram_parameter("masks", [128, kpc * 512], dt.bfloat16, isOutput=False)
    id_d = nc.declare_dram_parameter("ident", [128, 128], dt.bfloat16, isOutput=False)
    out_d = nc.declare_dram_parameter("out", [rows, D], dt.bfloat16, isOutput=True)

    with tile.TileContext(nc) as tc:
        with (
            tc.tile_pool(name="consts", bufs=1) as cp,
            tc.tile_pool(name="dram", bufs=1, space="DRAM") as dp,
        ):
            # ---- constants and small scalars ----
            # Ring order matters: ident (fillers) and the tiny scalars first,
            # then wq (first q MMs), then mask; wk/wv on the scalar ring.
            cosF = cp.tile([128, t], dt.bfloat16, tag="cosF")
            sinF = cp.tile([128, t], dt.bfloat16, tag="sinF")
            mask = cp.tile([128, kpc * 512], dt.bfloat16, tag="mask")
            ident = cp.tile([128, 128], dt.bfloat16, tag="ident")
            qkg = cp.tile([1, HQ], dt.float32, tag="qkg")
            lns = cp.tile([1, 1], dt.float32, tag="lns")
            vb = cp.tile([1, HD], dt.float32, tag="vb")
            nc.sync.dma_start(out=ident[:], in_=id_d[:])
            nc.sync.dma_start(out=qkg[:], in_=qkg_d[:])
            nc.sync.dma_start(out=lns[:], in_=lns_d[:])
            nc.sync.dma_start(out=vb[:], in_=vb_d[:])

            wkb = cp.tile([128, n_dt * HD], dt.bfloat16, tag="wkb")
            wvb = cp.tile([128, n_dt * HD], dt.bfloat16, tag="wvb")
            wq_sb = cp.tile([128, n_dt * HQ * HD], dt.bfloat16, tag="wq_sb")
            wraw = tc.alloc_tile_pool(name="wraw", bufs=1)
            wk_sb = wraw.tile([128, n_dt * HD], dt.bfloat16, tag="wk_sb")
            wv_sb = wraw.tile([128, n_dt * HD], dt.bfloat16, tag="wv_sb")
            # wq split per d-group so q(g0) can start before the full 1MB lands
            GD = 4  # d-tiles per DMA group
            n_grp = n_dt // GD
            for g_ in range(n_grp):
                nc.sync.dma_start(
                    out=wq_sb[:, GD * HQ * HD * g_ : GD * HQ * HD * (g_ + 1)].rearrange("p (k f) -> p k f", k=GD),
                    in_=wqT_d[128 * GD * g_ : 128 * GD * (g_ + 1), :].rearrange("(k p) f -> p k f", p=128),
                )
            nc.sync.dma_start(out=mask[:], in_=mask_d[:])
            nc.scalar.dma_start(
                out=wk_sb[:].rearrange("p (k f) -> p k f", k=n_dt),
                in_=wkT_d[:].rearrange("(k p) f -> p k f", p=128),
            )
            nc.scalar.dma_start(
                out=wv_sb[:].rearrange("p (k f) -> p k f", k=n_dt),
                in_=wvT_d[:].rearrange("(k p) f -> p k f", p=128),
            )

            ones_col = cp.tile([128, 1], dt.bfloat16, tag="ones_col")
            nc.gpsimd.memset(ones_col[:], 1.0)
            onef_row = cp.tile([1, 128], dt.float32, tag="onef_row")
            nc.gpsimd.memset(onef_row[:], 1.0)
            c15b = cp.tile([128, 512], dt.float32, tag="c15b")
            nc.gpsimd.memset(c15b[:], 1.5)

            # device scalars: g = sigmoid(0.1*lns) via exp so the only ACT
            # table set ever loaded is one containing exp (no Ln anywhere in
            # this kernel -> zero ACT_TABLE_LOAD swaps after the first).
            # All of this is emitted BEFORE the PE fillers: the pack matmuls
            # must not queue behind them (rbl128 gates every blend).
            emx = cp.tile([1, 1], dt.float32, tag="emx")
            nc.scalar.activation(emx[:], lns[:], AF.Exp, scale=-0.1)
            onep = cp.tile([1, 1], dt.float32, tag="onep")
            nc.vector.tensor_scalar_add(onep[:], emx[:], 1.0)
            g = cp.tile([1, 1], dt.float32, tag="g")
            nc.vector.reciprocal(g[:], onep[:])  # sigmoid
            omg = cp.tile([1, 1], dt.float32, tag="omg")
            nc.scalar.activation(omg[:], g[:], AF.Copy, bias=1.0, scale=-1.0)  # 1-g
            romg = cp.tile([1, 1], dt.float32, tag="romg")
            nc.vector.reciprocal(romg[:], omg[:])
            rblend = cp.tile([1, 1], dt.float32, tag="rblend")
            nc.vector.tensor_tensor(rblend[:], g[:], romg[:], ALU.mult)  # g/(1-g)
            gainsq = cp.tile([1, HQ], dt.float32, tag="gainsq")
            nc.vector.tensor_mul(gainsq[:], qkg[:], qkg[:])
            # gain^2/sqrt(HD) per head (folded into qT at write time), and
            # -Cmax = -sqrt(HD)*max_h gain^2 (softmax exp bias; head-indep)
            gsc2 = cp.tile([1, HQ], dt.float32, tag="gsc2")
            nc.vector.tensor_scalar_mul(gsc2[:], gainsq[:], float(1.0 / np.sqrt(HD)))
            gmax2 = cp.tile([1, 1], dt.float32, tag="gmax2")
            nc.vector.tensor_tensor(gmax2[:], gainsq[:, 0:1], gainsq[:, 1:2], ALU.max)
            # scalar pack: [negCmax, omg, lns, rblend] -> 4
            pack = cp.tile([1, 4], dt.float32, tag="pack")
            nc.scalar.activation(pack[:, 0:1], gmax2[:], AF.Copy, scale=-float(np.sqrt(HD)))
            nc.vector.tensor_copy(pack[:, 1:2], omg[:])
            nc.vector.tensor_copy(pack[:, 2:3], lns[:])
            nc.vector.tensor_copy(pack[:, 3:4], rblend[:])
            gsc3 = cp.tile([1, 3], dt.float32, tag="gsc3")
            nc.vector.tensor_copy(gsc3[:, 0:HQ], gsc2[:])
            nc.gpsimd.memset(gsc3[:, 2:3], 1.0)
            with tc.tile_pool(name="bc_ps", bufs=1, space="PSUM") as bcp:
                pk_ps = bcp.tile([128, 4], dt.float32, tag="pk_ps")
                nc.tensor.matmul(pk_ps[:], lhsT=onef_row[:], rhs=pack[:], start=True, stop=True)
                sc128 = cp.tile([128, 4], dt.float32, tag="sc128")
                nc.vector.tensor_copy(sc128[:], pk_ps[:])
                # per-unit scale rows for the rms broadcast matmul, placed at
                # partitions 0/32/64 (q0: g0^2/sqrt(HD), q1: g1^2/sqrt(HD),
                # k: 1.0) so lhsT/rhs/out base-partition rules line up with
                # the rsqrt rows living at partitions 0/32/64.
                gq_ps = bcp.tile([128, 128], dt.float32, tag="gq_ps")
                for u in range(3):
                    nc.tensor.matmul(gq_ps[32 * u : 32 * u + 1, :], lhsT=gsc3[:, u : u + 1], rhs=onef_row[:], start=True, stop=True)
                growq = cp.tile([128, 128], dt.float32, tag="growq")
                for u in range(3):
                    nc.vector.tensor_copy(growq[32 * u : 32 * u + 1, :], gq_ps[32 * u : 32 * u + 1, :])
            negCmax = sc128[:, 0:1]
            omg128 = sc128[:, 1:2]
            lns128 = sc128[:, 2:3]
            rbl128 = sc128[:, 3:4]
            # (1-g)-scaled k/v weights (DVE; waits only on wk/wv + omg128)
            nc.vector.tensor_scalar_mul(wkb[:], wk_sb[:], omg128[:, 0:1])
            nc.vector.tensor_scalar_mul(wvb[:], wv_sb[:], omg128[:, 0:1])

            # PE fillers AFTER the pack matmuls: dependency-chained dummy MMs
            # keep the HAM clock warm through the initial x-DMA window.
            fillp = tc.alloc_tile_pool(name="fill_ps", bufs=1, space="PSUM")
            fps = fillp.tile([128, 128], dt.float32, tag="fill")
            for i in range(60):
                nc.tensor.matmul(fps[:], lhsT=ident[:], rhs=ident[:], start=(i == 0), stop=(i == 59))

            # scaled v_bias (1-g)*v_bias, transposed to [128,1] via PE
            # (emitted after the fillers: its input chain lands ~5us in)
            vbs = cp.tile([1, HD], dt.float32, tag="vbs")
            nc.vector.tensor_scalar_mul(vbs[:], vb[:], omg[:, 0:1])
            vbsT = cp.tile([128, 1], dt.float32, tag="vbsT")
            with tc.tile_pool(name="bc2_ps", bufs=1, space="PSUM") as bcp2:
                vb_ps = bcp2.tile([128, 1], dt.float32, tag="vb_ps")
                nc.tensor.matmul(vb_ps[:], lhsT=vbs[:], rhs=onef_row[:, 0:1], start=True, stop=True)
                nc.vector.tensor_copy(vbsT[:], vb_ps[:])

            # ---- persistent activations (raw proj, then normed in place) ----
            kT = cp.tile([128, t], dt.bfloat16, tag="kT")
            vT_sb = cp.tile([128, t], dt.bfloat16, tag="vT_sb")
            vnat = cp.tile([128, t], dt.bfloat16, tag="vnat")
            qT = cp.tile([128, HQ * t], dt.bfloat16, tag="qT")

            # ---- phase 1: group-major over 512-token columns ----
            # PE starts on q(g0) as soon as the first 0.5MB x group lands;
            # k/v trail one d-group behind their gpsimd blends. Per-column
            # epilogues (square/rowsum -> rsqrt chain -> broadcast+RoPE) are
            # deferred into the next column\'s MM stream.
            fillp.release()
            n_col = t // 512
            with (
                tc.tile_pool(name="xin", bufs=10) as xp,
                tc.tile_pool(name="skin", bufs=6) as skp,
                tc.tile_pool(name="stbp", bufs=20) as sbp,
                tc.tile_pool(name="p1s", bufs=2) as sp,
                tc.tile_pool(name="p1ps", bufs=1, space="PSUM") as psp,
                tc.tile_pool(name="p1row", bufs=2, space="PSUM") as rowp,
                tc.tile_pool(name="p1rb", bufs=1, space="PSUM") as rbp,
                tc.tile_pool(name="vtrp", bufs=1, space="PSUM") as vtrp,
            ):
                # rms_norm rsqrt via bit-trick ln + exp-table + 2 Newton steps
                # (keeps the ACT table on the exp set: no Ln -> no table swaps)
                LN2 = float(np.log(2.0))
                RS_SCALE = -0.5 * LN2 / (1 << 23)
                RS_BIAS = 0.5 * LN2 * (127 + 0.0450466) + 0.5 * float(np.log(HD))
                MINBITS = int(np.float32(1e-6).view(np.int32))
                rsb = cp.tile([128, 1], dt.float32, tag="rsb")
                nc.gpsimd.memset(rsb[:], RS_BIAS)

                deferred = []

                def pop_deferred(n=1):
                    for _ in range(n):
                        if deferred:
                            deferred.pop(0)()

                xg = {}
                sk = {}
                stb = {}

                def emit_col_dmas(c):
                    for g_ in range(n_grp):
                        xx = xp.tile([128, GD * 512], dt.bfloat16, tag="xg")
                        engx = nc.sync if (c + g_) % 2 == 0 else nc.scalar
                        engx.dma_start(
                            out=xx[:].rearrange("p (k f) -> p k f", k=GD),
                            in_=xT_d[128 * GD * g_ : 128 * GD * (g_ + 1), 512 * c : 512 * (c + 1)].rearrange("(k p) f -> p k f", p=128),
                        )
                        xg[(c, g_)] = xx
                        ss = skp.tile([128, GD * 512], dt.bfloat16, tag="sg")
                        engs = nc.scalar if (c + g_) % 2 == 0 else nc.sync
                        engs.dma_start(
                            out=ss[:].rearrange("p (k f) -> p k f", k=GD),
                            in_=skT_d[128 * GD * g_ : 128 * GD * (g_ + 1), 512 * c : 512 * (c + 1)].rearrange("(k p) f -> p k f", p=128),
                        )
                        sk[(c, g_)] = ss

                def emit_blends(c, g_):
                    # stb = x + (g/(1-g))*skip (DVE scalar_tensor_tensor;
                    # TensorScalarPtr is illegal on the Pool engine)
                    for kk in range(GD):
                        k_ = GD * g_ + kk
                        bl = sbp.tile([128, 512], dt.bfloat16, tag="stb")
                        nc.vector.scalar_tensor_tensor(
                            bl[:], sk[(c, g_)][:, 512 * kk : 512 * (kk + 1)], rbl128[:, 0:1],
                            xg[(c, g_)][:, 512 * kk : 512 * (kk + 1)], ALU.mult, ALU.add,
                        )
                        stb[(c, k_)] = bl

                def unit_mms(c, kind, g_, col_ps):
                    ps = col_ps[kind]
                    for j in range(GD):
                        k_ = GD * g_ + j
                        if kind == "k" or kind == "v":
                            w = wkb if kind == "k" else wvb
                            lhsT = w[:, k_ * HD : (k_ + 1) * HD]
                            rhs = stb[(c, k_)][:]
                        else:
                            h = int(kind[1])
                            lhsT = wq_sb[:, k_ * HQ * HD + h * HD : k_ * HQ * HD + (h + 1) * HD]
                            rhs = xg[(c, g_)][:, 512 * j : 512 * (j + 1)]
                        nc.tensor.matmul(ps[:], lhsT=lhsT, rhs=rhs, start=(k_ == 0), stop=(k_ == n_dt - 1))

                def col_sq_ssq(raw_cs, rowps, u):
                    sq = sp.tile([128, 512], dt.bfloat16, tag=f"sq{u}")
                    nc.scalar.square(sq[:], raw_cs)
                    nc.tensor.matmul(rowps[32 * u : 32 * u + 1, :], lhsT=ones_col[:], rhs=sq[:], start=True, stop=True)

                def rsq_chain(rowps):
                    """y2[32u,:] ~= rsqrt(rowps[32u,:]/HD), all rows at once.
                    Seed on DVE/ACT; the two Newton steps run as TT-only ops
                    on the otherwise-idle gpsimd (no PSUM reads there)."""
                    A = sp.tile([128, 512], dt.float32, tag="rsA")
                    B = sp.tile([128, 512], dt.float32, tag="rsB")
                    S = sp.tile([128, 512], dt.float32, tag="rsS")
                    nc.vector.tensor_scalar(S[:], rowps[:], 0.5 / HD, None, ALU.mult)
                    nc.vector.tensor_scalar(A[:], rowps[:].bitcast(dt.int32), MINBITS, None, ALU.max)
                    nc.scalar.activation(B[:], A[:], AF.Exp, bias=rsb[:, 0:1], scale=RS_SCALE)
                    nc.gpsimd.tensor_tensor(A[:], B[:], B[:], ALU.mult)
                    nc.gpsimd.tensor_tensor(A[:], A[:], S[:], ALU.mult)
                    nc.gpsimd.tensor_tensor(A[:], c15b[:], A[:], ALU.subtract)
                    nc.gpsimd.tensor_tensor(A[:], B[:], A[:], ALU.mult)  # A = y1
                    nc.gpsimd.tensor_tensor(B[:], A[:], A[:], ALU.mult)
                    nc.gpsimd.tensor_tensor(B[:], B[:], S[:], ALU.mult)
                    nc.gpsimd.tensor_tensor(B[:], c15b[:], B[:], ALU.subtract)
                    nc.gpsimd.tensor_tensor(B[:], A[:], B[:], ALU.mult)  # B = y2
                    return B

                def col_part2(raw_cs, c, y2, u):
                    """PE broadcast (per-head gain^2/sqrt(HD) folded into the
                    q lhsT rows of growq) + normalize + RoPE, one column."""
                    cs = slice(512 * c, 512 * (c + 1))
                    qh = sp.tile([128, 512], dt.bfloat16, tag="qh")
                    rb = rbp.tile([128, 512], dt.float32, tag="rb")
                    nc.tensor.matmul(rb[:], lhsT=growq[32 * u : 32 * u + 1, :], rhs=y2[32 * u : 32 * u + 1, :], start=True, stop=True)
                    nc.vector.tensor_mul(qh[:], raw_cs, rb[:])
                    qsw = sp.tile([128, 512], dt.bfloat16, tag="qsw")
                    nc.gpsimd.tensor_copy(qsw[0:64, :], qh[64:128, :])
                    nc.gpsimd.tensor_copy(qsw[64:128, :], qh[0:64, :])
                    tsw = sp.tile([128, 512], dt.bfloat16, tag="tsw")
                    nc.vector.tensor_mul(tsw[:], qsw[:], sinF[:, cs])
                    tco = sp.tile([128, 512], dt.bfloat16, tag="tco")
                    nc.vector.tensor_mul(tco[:], qh[:], cosF[:, cs])
                    nc.vector.tensor_add(raw_cs, tco[:], tsw[:])

                def vnat_transpose(c):
                    vtp = vtrp.tile([128, 512], dt.bfloat16, tag="vtr")
                    for s_ in range(4):
                        kt = 4 * c + s_
                        nc.tensor.transpose(vtp[:, 128 * s_ : 128 * (s_ + 1)], vT_sb[:, 128 * kt : 128 * (kt + 1)], ident[:])
                    nc.vector.tensor_copy(vnat[:, 512 * c : 512 * (c + 1)], vtp[:])

                emit_col_dmas(0)
                nc.sync.dma_start(out=cosF[:], in_=cosF_d[:])
                nc.scalar.dma_start(out=sinF[:], in_=sinF_d[:])
                for c in range(n_col):
                    if c + 1 < n_col:
                        emit_col_dmas(c + 1)
                    col_ps = {kd: psp.tile([128, 512], dt.float32, tag=f"ps_{kd}", name=f"ps_{kd}") for kd in ("q0", "q1", "k", "v")}
                    for g_ in range(n_grp):
                        pop_deferred(2)
                        unit_mms(c, "q0", g_, col_ps)
                        unit_mms(c, "q1", g_, col_ps)
                        emit_blends(c, g_)
                        if g_ > 0:
                            unit_mms(c, "k", g_ - 1, col_ps)
                            unit_mms(c, "v", g_ - 1, col_ps)
                    pop_deferred(1)
                    unit_mms(c, "k", n_grp - 1, col_ps)
                    unit_mms(c, "v", n_grp - 1, col_ps)
                    # drains (q/k on ACT which is idle here; v adds its bias)
                    cs = slice(512 * c, 512 * (c + 1))
                    q0_cs = qT[:, t * 0 + 512 * c : t * 0 + 512 * (c + 1)]
                    q1_cs = qT[:, t * 1 + 512 * c : t * 1 + 512 * (c + 1)]
                    k_cs = kT[:, cs]
                    nc.scalar.activation(q0_cs, col_ps["q0"][:], AF.Copy)
                    nc.scalar.activation(q1_cs, col_ps["q1"][:], AF.Copy)
                    nc.scalar.activation(k_cs, col_ps["k"][:], AF.Copy)
                    nc.vector.tensor_scalar_add(vT_sb[:, cs], col_ps["v"][:], vbsT[:, 0:1])
                    # defer the normalization epilogue into the next column
                    rowps = rowp.tile([128, 512], dt.float32, tag="rowps")
                    y2h = {}
                    raws = {0: q0_cs, 1: q1_cs, 2: k_cs}

                    def mk_sq(u, raws=raws, rowps=rowps):
                        def f():
                            col_sq_ssq(raws[u], rowps, u)
                        return f

                    def mk_chain(raws=raws, rowps=rowps, y2h=y2h):
                        def f():
                            col_sq_ssq(raws[2], rowps, 2)
                            y2h[0] = rsq_chain(rowps)
                        return f

                    def mk_p2(u, c=c, raws=raws, y2h=y2h):
                        def f():
                            col_part2(raws[u], c, y2h[0], u)
                        return f

                    deferred.append(mk_sq(0))
                    deferred.append(mk_sq(1))
                    deferred.append(mk_chain())
                    deferred.append(mk_p2(0))
                    deferred.append(mk_p2(1))
                    deferred.append(mk_p2(2))
                    deferred.append(lambda c=c: vnat_transpose(c))

                # drain remaining deferred work
                for d in deferred:
                    d()
                deferred = []
            wraw.release()

            # ---- phase 2: attention ----
            y_in = [dp.tile([NCORES, HD, rows], dt.bfloat16, name=f"y_in{h}", tag=f"y_in{h}") for h in range(HQ)]
            y_out = [dp.tile([NCORES, HD, rows], dt.bfloat16, name=f"y_out{h}", tag=f"y_out{h}") for h in range(HQ)]

            # wproj prefetch (full, both HWDGE rings) during phase 2;
            # one 2 MB coalesced DMA per column block
            prp = tc.alloc_tile_pool(name="pr_s", bufs=4)
            wps = {}
            for n in range(D // 512):
                wp = prp.tile([128, n_dt * 512], dt.bfloat16, name=f"wp{n}", tag="wp")
                eng = nc.sync if n % 2 == 0 else nc.scalar
                eng.dma_start(
                    out=wp[:].rearrange("p (k f) -> p k f", k=n_dt),
                    in_=wpT_d[:, 512 * n : 512 * (n + 1)].rearrange("(k p) f -> p k f", p=128),
                )
                wps[n] = wp

            ytp_ = tc.alloc_tile_pool(name="yt_s", bufs=1)
            yt_blocks = [None] * n_dt
            with (
                tc.tile_pool(name="att_s", bufs=5) as ap_,
                tc.tile_pool(name="acc_s", bufs=2) as ap2,
                tc.tile_pool(name="st_ps", bufs=3, space="PSUM") as stp_,
                tc.tile_pool(name="yl_ps", bufs=2, space="PSUM") as ylp_,
            ):
                def epi_part1(h, c, ytp, accA, accB):
                    # l = ones.T @ (accA + accB); 1/l on the DVE (table-free)
                    lrow_t = stp_.tile([128, 1024], dt.float32, tag="st", name="lrow_t")
                    lrow = lrow_t[0:1, 0:512]
                    nc.tensor.matmul(lrow, lhsT=ones_col[:], rhs=accA[:], start=True, stop=False)
                    nc.tensor.matmul(lrow, lhsT=ones_col[:], rhs=accB[:], start=False, stop=True)
                    rl = ap_.tile([1, 512], dt.float32, tag="rl")
                    nc.vector.reciprocal_approx_fast(out=rl[:], in_=lrow)
                    return (h, c, ytp, rl)

                def epi_part2(h, c, ytp, rl):
                    # broadcast 1/l, normalize, ship pieces
                    rb2_t = stp_.tile([128, 1024], dt.float32, tag="st", name="rb2_t")
                    rb2 = rb2_t[:, 0:512]
                    nc.tensor.matmul(rb2, lhsT=onef_row[:], rhs=rl[:], start=True, stop=True)
                    rb2s = ap_.tile([128, 512], dt.float32, tag="rb2s")
                    nc.vector.tensor_copy(rb2s[:], rb2)
                    ysb = ap_.tile([128, 512], dt.bfloat16, tag="ysb")
                    nc.vector.tensor_mul(ysb[:], ytp[:], rb2s[:])
                    for b in range(512 // rows):
                        piece = (512 * c) // rows + b
                        nc.sync.dma_start(
                            out=y_in[h][piece, :, :],
                            in_=ysb[:, rows * b : rows * (b + 1)],
                        )

                prev_epi = None  # (h, c, ytp, acc): chunk awaiting part1
                epi1 = None  # (h, c, ytp, rl): awaiting part2
                for h in range(HQ):
                    for c in range(n_ch):
                        qs = slice(t * h + 512 * c, t * h + 512 * (c + 1))
                        nkts = kpc * (c + 1)
                        ytp = ylp_.tile([128, 512], dt.float32, tag="yt")
                        # two alternating accumulators halve the serial DVE
                        # dependency chain for the softmax denominator
                        accA = ap2.tile([128, 512], dt.bfloat16, tag="accA")
                        accB = ap2.tile([128, 512], dt.bfloat16, tag="accB")
                        pend = []  # [(pp, kts)] awaiting y/acc emission (2-deep)

                        def emit_pend(p, last, ytp=ytp, accA=accA, accB=accB):
                            ppp, kts_ = p
                            for s_, kt_ in enumerate(kts_):
                                pseg = ppp[:, 512 * s_ : 512 * (s_ + 1)]
                                nc.tensor.matmul(ytp[:], lhsT=vnat[:, HD * kt_ : HD * (kt_ + 1)], rhs=pseg, start=(kt_ == 0), stop=(last and kt_ == kts_[-1]))
                                acc = accA if kt_ % 2 == 0 else accB
                                if kt_ < 2:
                                    nc.vector.tensor_copy(acc[:], pseg)
                                else:
                                    nc.vector.tensor_add(acc[:], acc[:], pseg)

                        for pgi in range(nkts // 2):
                            kts = [2 * pgi, 2 * pgi + 1]
                            stp = stp_.tile([128, 1024], dt.float32, tag="st")
                            for s, kt in enumerate(kts):
                                seg = stp[:, 512 * s : 512 * (s + 1)]
                                diag = kt >= kpc * c
                                nc.tensor.matmul(seg, lhsT=kT[:, 128 * kt : 128 * (kt + 1)], rhs=qT[:, qs], start=True, stop=not diag)
                                if diag:
                                    m = kt - kpc * c
                                    nc.tensor.matmul(seg, lhsT=ident[:], rhs=mask[:, 512 * m : 512 * (m + 1)], start=False, stop=True)
                            # 2-deep lookahead: y/acc for group i-2 land after
                            # scores of group i, so exp latency is fully hidden
                            if len(pend) >= 2:
                                emit_pend(pend.pop(0), last=False)
                            pp = ap_.tile([128, 1024], dt.bfloat16, tag="pp")
                            nc.scalar.activation(pp[:], stp[:], AF.Exp, bias=negCmax[:, 0:1], scale=1.0)
                            npg = nkts // 2
                            p1_at = min(npg - 2, 3) if npg >= 3 else 0
                            if pgi == p1_at and prev_epi is not None:
                                epi1 = epi_part1(*prev_epi)
                                prev_epi = None
                            elif pgi == p1_at + 1 and epi1 is not None:
                                epi_part2(*epi1)
                                epi1 = None
                            pend.append((pp, kts))
                        while pend:
                            emit_pend(pend.pop(0), last=(len(pend) == 0))
                        prev_epi = (h, c, ytp, accA, accB)
                    # flush last chunk's epilogue before the collective
                    epi_part2(*epi_part1(*prev_epi))
                    prev_epi = None
                    nc.gpsimd.collective_compute(
                        "AllToAll",
                        ALU.bypass,
                        replica_groups=[list(range(NCORES))],
                        ins=[y_in[h].opt()],
                        outs=[y_out[h].opt()],
                    )
                    if h == 0:
                        yb = ytp_.tile([128, NCORES * rows], dt.bfloat16, name="ytall0", tag="ytall0")
                        nc.sync.dma_start(
                            out=yb[:].rearrange("p (j r) -> p j r", j=NCORES),
                            in_=y_out[0][:].rearrange("j p r -> p j r"),
                        )
                        for j in range(NCORES):
                            yt_blocks[2 * j] = yb[:, rows * j : rows * (j + 1)]

            # ---- phase 3: output projection (h0 pass overlaps A2A(h1)) ----
            yb1 = ytp_.tile([128, NCORES * rows], dt.bfloat16, name="ytall1", tag="ytall1")
            nc.sync.dma_start(
                out=yb1[:].rearrange("p (j r) -> p j r", j=NCORES),
                in_=y_out[1][:].rearrange("j p r -> p j r"),
            )
            for j in range(NCORES):
                yt_blocks[2 * j + 1] = yb1[:, rows * j : rows * (j + 1)]

            mb = min(128, rows)
            nb = rows // mb
            tiles3 = [(n, b) for n in range(D // 512) for b in range(nb)]
            with (
                tc.tile_pool(name="pr_ps", bufs=1, space="PSUM") as prps,
                tc.tile_pool(name="pr_out", bufs=2) as prout,
            ):
                opss = {}
                for (n, b) in tiles3:
                    ops = prps.tile([mb, 512], dt.float32, tag=f"ops{n}_{b}")
                    opss[(n, b)] = ops
                    for ai, a in enumerate(range(0, n_dt, 2)):  # h0 blocks
                        nc.tensor.matmul(
                            ops[:],
                            lhsT=yt_blocks[a][:, mb * b : mb * (b + 1)],
                            rhs=wps[n][:, 512 * a : 512 * (a + 1)],
                            start=(ai == 0),
                            stop=False,
                        )
                for (n, b) in tiles3:
                    ops = opss[(n, b)]
                    for ai, a in enumerate(range(1, n_dt, 2)):  # h1 blocks
                        nc.tensor.matmul(
                            ops[:],
                            lhsT=yt_blocks[a][:, mb * b : mb * (b + 1)],
                            rhs=wps[n][:, 512 * a : 512 * (a + 1)],
                            start=False,
                            stop=(ai == n_dt // 2 - 1),
                        )
                    osb = prout.tile([mb, 512], dt.bfloat16, tag="osb")
                    nc.scalar.activation(osb[:], ops[:], AF.Copy, scale=lns128[:mb, 0:1])
                    nc.sync.dma_start(
                        out=out_d[mb * b : mb * (b + 1), 512 * n : 512 * (n + 1)],
                        in_=osb[:],
                    )
            ytp_.release()
            prp.release()
    nc.finalize()
    return nc


def make_tables(t=T):
    pos = np.arange(t, dtype=np.float32)
    inv = 1.0 / (ROPE_BASE ** (np.arange(0, HD, 2, dtype=np.float32) / HD))
    fr = pos[:, None] * inv[None, :]  # [t, 64]
    cos = np.cos(fr).T  # [64, t]
    sin = np.sin(fr).T
    cosF = np.concatenate([cos, cos], axis=0)  # [128, t]
    sinF = np.concatenate([sin, -sin], axis=0)
    return _bf(cosF), _bf(sinF)


def make_masks():
    # mask[p, 512*m + j] = 0 if j >= 128*m + p else MASK_VAL
    p = np.arange(128)[:, None]
    j = np.arange(512)[None, :]
    blocks = [np.where(j >= 128 * m + p, 0.0, MASK_VAL) for m in range(4)]
    return _bf(np.concatenate(blocks, axis=1))


_GRAPH_CACHE = {}
_LAST_IN_MAPS = None


def kernel(x, skip, wq, wk, wv, wproj, qk_g, ln_s, v_bias):
    t = x.shape[1]
    if t not in _GRAPH_CACHE:
        _GRAPH_CACHE[t] = build_graph(t)
    nc = _GRAPH_CACHE[t]

    xT = _bf(x.reshape(t, D).T)
    skT = _bf(skip.reshape(t, D).T)
    wpT = _bf(np.asarray(wproj, np.float32).T)
    cosF, sinF = make_tables(t)
    masks = make_masks()
    ident = _bf(np.eye(128, dtype=np.float32))

    in_maps = []
    for c in range(NCORES):
        kv = c // 2
        in_maps.append(
            {
                "xT": xT,
                "skipT": skT,
                "wqT": _bf(np.asarray(wq, np.float32)[HQ * HD * c : HQ * HD * (c + 1), :].T),
                "wkT": _bf(np.asarray(wk, np.float32)[HD * kv : HD * (kv + 1), :].T),
                "wvT": _bf(np.asarray(wv, np.float32)[HD * kv : HD * (kv + 1), :].T),
                "wprojT": wpT,
                "qkg": np.asarray(qk_g, np.float32)[HQ * c : HQ * (c + 1)].reshape(1, HQ),
                "lns": np.asarray(ln_s, np.float32).reshape(1, 1),
                "vbias": np.asarray(v_bias, np.float32)[kv].reshape(1, HD),
                "cosF": cosF,
                "sinF": sinF,
                "masks": masks,
                "ident": ident,
            }
        )
    global _LAST_IN_MAPS
    _LAST_IN_MAPS = in_maps
    res = run_bass_kernel_spmd(nc, in_maps, list(range(NCORES)))
    out = np.concatenate(
        [np.asarray(res.results[c]["out"], np.float32) for c in range(NCORES)], axis=0
    )
    return out.reshape(1, t, D).astype(np.float32)



# revision 26
# speedup vs baseline: 1.1878x; 1.1878x over previous
"""Distributed Trainium2 kernel for nn_Attention_25228637897408.

GQA attention (B=1, T=2048, D=2048, NH=16, NKV=4, HD=128) with RoPE,
per-head rms_norm, skip-gate blend of k/v, v_bias, causal softmax and
output projection, tensor-parallel over heads on 8 NeuronCores.

Per-core work (core c):
  - q-heads {2c, 2c+1}, kv-head c//2.
  - skip blend done ONCE on activations: stb = x + (g/(1-g))*skip (DVE
    scalar_tensor_tensor); k/v projections contract stb against
    (1-g)-scaled weights -> halves the k/v matmul count.
  - phase 1 streams x/skip in 1024-token halves; each projection unit
    is a dense 32-MM burst whose raw result is copied straight into the
    persistent kT/qT tiles; rms_norm+RoPE epilogues are batched per
    1024-token row and interleaved into later MM bursts so the PE never
    starves (keeps the HAM clock warm).
  - partition reductions (sum-of-squares, softmax denominator) are
    single ones-column matmuls into [1,512] PSUM rows; 1/x and
    1/sqrt(x) via exp(-ln(x)) on ACT: the whole kernel uses one
    activation table set (natural_log_exp_and_others).
  - attention per 128-k-tile: scores -> exp -> y/l accumulate, emitted
    software-pipelined (scores(kt+1) ahead of y/l(kt)) so exp latency
    hides under PE work; causal mask added on the PE via identity x
    mask matmul; exp with per-head scale=gain^2/sqrt(HD) and
    bias=-gain^2*sqrt(HD).
  - AllToAll (gpsimd stays free of DMA work so triggers fire promptly)
    redistributes y; the output projection is split into an h0-block
    pass that overlaps A2A(h1)'s flight and an h1-block pass (8 PSUM
    banks held across the passes).
Host side only reshapes/transposes/casts and slices inputs; all value
computation (sigmoid, blending, norms, softmax, matmuls) is on device.
"""

import sys

sys.path.insert(0, "/opt/trn_rl_repo")

import numpy as np
import ml_dtypes

import concourse.bass as bass
import concourse.mybir as mybir
import concourse.tile as tile
from concourse import bacc
from concourse.bass_utils import run_bass_kernel_spmd

BF16 = ml_dtypes.bfloat16

T = 2048
D = 2048
NH = 16
NKV = 4
HD = 128
REP = NH // NKV
NCORES = 8
HQ = NH // NCORES  # q heads per core = 2
ROPE_BASE = 10000.0
EPS = float(np.finfo(np.float32).eps)
MASK_VAL = -1.0e5

dt = mybir.dt
AF = mybir.ActivationFunctionType
ALU = mybir.AluOpType


def _bf(x):
    return np.ascontiguousarray(np.asarray(x, dtype=np.float32)).astype(BF16)


def build_graph(t=T):
    """Build the SPMD graph (identical on all cores). t parametrizes the
    sequence length for simulator-sized testing."""
    assert t % 1024 == 0
    n_ch = t // 512  # 512-wide token chunks
    n_kt = t // 128  # 128-row tiles along T
    kpc = n_kt // n_ch  # k-tiles per chunk = 4
    rows = t // NCORES  # output rows per core
    n_dt = D // 128  # tiles along D contraction = 16
    HW = 1024  # phase-1 half width
    n_half = t // HW

    nc = bacc.Bacc(None, target_bir_lowering=False)

    xT_d = nc.declare_dram_parameter("xT", [D, t], dt.bfloat16, isOutput=False)
    skT_d = nc.declare_dram_parameter("skipT", [D, t], dt.bfloat16, isOutput=False)
    wqT_d = nc.declare_dram_parameter("wqT", [D, HQ * HD], dt.bfloat16, isOutput=False)
    wkT_d = nc.declare_dram_parameter("wkT", [D, HD], dt.bfloat16, isOutput=False)
    wvT_d = nc.declare_dram_parameter("wvT", [D, HD], dt.bfloat16, isOutput=False)
    wpT_d = nc.declare_dram_parameter("wprojT", [D, D], dt.bfloat16, isOutput=False)
    qkg_d = nc.declare_dram_parameter("qkg", [1, HQ], dt.float32, isOutput=False)
    lns_d = nc.declare_dram_parameter("lns", [1, 1], dt.float32, isOutput=False)
    vb_d = nc.declare_dram_parameter("vbias", [1, HD], dt.float32, isOutput=False)
    cosF_d = nc.declare_dram_parameter("cosF", [HD, t], dt.bfloat16, isOutput=False)
    sinF_d = nc.declare_dram_parameter("sinF", [HD, t], dt.bfloat16, isOutput=False)
    mask_d = nc.declare_dram_parameter("masks", [128, kpc * 512], dt.bfloat16, isOutput=False)
    id_d = nc.declare_dram_parameter("ident", [128, 128], dt.bfloat16, isOutput=False)
    out_d = nc.declare_dram_parameter("out", [rows, D], dt.bfloat16, isOutput=True)

    with tile.TileContext(nc) as tc:
        with (
            tc.tile_pool(name="consts", bufs=1) as cp,
            tc.tile_pool(name="dram", bufs=1, space="DRAM") as dp,
        ):
            # ---- constants and small scalars ----
            # Ring order matters: ident (fillers) and the tiny scalars first,
            # then wq (first q MMs), then mask; wk/wv on the scalar ring.
            cosF = cp.tile([128, t], dt.bfloat16, tag="cosF")
            sinF = cp.tile([128, t], dt.bfloat16, tag="sinF")
            mask = cp.tile([128, kpc * 512], dt.bfloat16, tag="mask")
            ident = cp.tile([128, 128], dt.bfloat16, tag="ident")
            qkg = cp.tile([1, HQ], dt.float32, tag="qkg")
            lns = cp.tile([1, 1], dt.float32, tag="lns")
            vb = cp.tile([1, HD], dt.float32, tag="vb")
            nc.sync.dma_start(out=ident[:], in_=id_d[:])
            nc.sync.dma_start(out=qkg[:], in_=qkg_d[:])
            nc.sync.dma_start(out=lns[:], in_=lns_d[:])
            nc.sync.dma_start(out=vb[:], in_=vb_d[:])

            wkb = cp.tile([128, n_dt * HD], dt.bfloat16, tag="wkb")
            wvb = cp.tile([128, n_dt * HD], dt.bfloat16, tag="wvb")
            wq_sb = cp.tile([128, n_dt * HQ * HD], dt.bfloat16, tag="wq_sb")
            wraw = tc.alloc_tile_pool(name="wraw", bufs=1)
            wk_sb = wraw.tile([128, n_dt * HD], dt.bfloat16, tag="wk_sb")
            wv_sb = wraw.tile([128, n_dt * HD], dt.bfloat16, tag="wv_sb")
            # wq split per d-group so q(g0) can start before the full 1MB lands
            GD = 4  # d-tiles per DMA group
            n_grp = n_dt // GD
            for g_ in range(n_grp):
                nc.sync.dma_start(
                    out=wq_sb[:, GD * HQ * HD * g_ : GD * HQ * HD * (g_ + 1)].rearrange("p (k f) -> p k f", k=GD),
                    in_=wqT_d[128 * GD * g_ : 128 * GD * (g_ + 1), :].rearrange("(k p) f -> p k f", p=128),
                )
            nc.sync.dma_start(out=mask[:], in_=mask_d[:])
            nc.scalar.dma_start(
                out=wk_sb[:].rearrange("p (k f) -> p k f", k=n_dt),
                in_=wkT_d[:].rearrange("(k p) f -> p k f", p=128),
            )
            nc.scalar.dma_start(
                out=wv_sb[:].rearrange("p (k f) -> p k f", k=n_dt),
                in_=wvT_d[:].rearrange("(k p) f -> p k f", p=128),
            )

            ones_col = cp.tile([128, 1], dt.bfloat16, tag="ones_col")
            nc.gpsimd.memset(ones_col[:], 1.0)
            onef_row = cp.tile([1, 128], dt.float32, tag="onef_row")
            nc.gpsimd.memset(onef_row[:], 1.0)
            c15s = cp.tile([128, 1], dt.float32, tag="c15s")
            nc.gpsimd.memset(c15s[:], 1.5)

            # device scalars: g = sigmoid(0.1*lns) via exp so the only ACT
            # table set ever loaded is one containing exp (no Ln anywhere in
            # this kernel -> zero ACT_TABLE_LOAD swaps after the first).
            # All of this is emitted BEFORE the PE fillers: the pack matmuls
            # must not queue behind them (rbl128 gates every blend).
            emx = cp.tile([1, 1], dt.float32, tag="emx")
            nc.scalar.activation(emx[:], lns[:], AF.Exp, scale=-0.1)
            onep = cp.tile([1, 1], dt.float32, tag="onep")
            nc.vector.tensor_scalar_add(onep[:], emx[:], 1.0)
            g = cp.tile([1, 1], dt.float32, tag="g")
            nc.vector.reciprocal(g[:], onep[:])  # sigmoid
            omg = cp.tile([1, 1], dt.float32, tag="omg")
            nc.scalar.activation(omg[:], g[:], AF.Copy, bias=1.0, scale=-1.0)  # 1-g
            romg = cp.tile([1, 1], dt.float32, tag="romg")
            nc.vector.reciprocal(romg[:], omg[:])
            rblend = cp.tile([1, 1], dt.float32, tag="rblend")
            nc.vector.tensor_tensor(rblend[:], g[:], romg[:], ALU.mult)  # g/(1-g)
            gainsq = cp.tile([1, HQ], dt.float32, tag="gainsq")
            nc.vector.tensor_mul(gainsq[:], qkg[:], qkg[:])
            # gain^2/sqrt(HD) per head (folded into qT at write time), and
            # -Cmax = -sqrt(HD)*max_h gain^2 (softmax exp bias; head-indep)
            gsc2 = cp.tile([1, HQ], dt.float32, tag="gsc2")
            nc.vector.tensor_scalar_mul(gsc2[:], gainsq[:], float(1.0 / np.sqrt(HD)))
            gmax2 = cp.tile([1, 1], dt.float32, tag="gmax2")
            nc.vector.tensor_tensor(gmax2[:], gainsq[:, 0:1], gainsq[:, 1:2], ALU.max)
            # scalar pack: [negCmax, omg, lns, rblend] -> 4
            pack = cp.tile([1, 4], dt.float32, tag="pack")
            nc.scalar.activation(pack[:, 0:1], gmax2[:], AF.Copy, scale=-float(np.sqrt(HD)))
            nc.vector.tensor_copy(pack[:, 1:2], omg[:])
            nc.vector.tensor_copy(pack[:, 2:3], lns[:])
            nc.vector.tensor_copy(pack[:, 3:4], rblend[:])
            gsc3 = cp.tile([1, 3], dt.float32, tag="gsc3")
            nc.vector.tensor_copy(gsc3[:, 0:HQ], gsc2[:])
            nc.gpsimd.memset(gsc3[:, 2:3], 1.0)
            with tc.tile_pool(name="bc_ps", bufs=1, space="PSUM") as bcp:
                pk_ps = bcp.tile([128, 4], dt.float32, tag="pk_ps")
                nc.tensor.matmul(pk_ps[:], lhsT=onef_row[:], rhs=pack[:], start=True, stop=True)
                sc128 = cp.tile([128, 4], dt.float32, tag="sc128")
                nc.vector.tensor_copy(sc128[:], pk_ps[:])
                # per-unit scale rows for the rms broadcast matmul, placed at
                # partitions 0/32/64 (q0: g0^2/sqrt(HD), q1: g1^2/sqrt(HD),
                # k: 1.0) so lhsT/rhs/out base-partition rules line up with
                # the rsqrt rows living at partitions 0/32/64.
                gq_ps = bcp.tile([128, 128], dt.float32, tag="gq_ps")
                for u in range(3):
                    nc.tensor.matmul(gq_ps[32 * u : 32 * u + 1, :], lhsT=gsc3[:, u : u + 1], rhs=onef_row[:], start=True, stop=True)
                growq = cp.tile([128, 128], dt.float32, tag="growq")
                for u in range(3):
                    nc.vector.tensor_copy(growq[32 * u : 32 * u + 1, :], gq_ps[32 * u : 32 * u + 1, :])
            negCmax = sc128[:, 0:1]
            omg128 = sc128[:, 1:2]
            lns128 = sc128[:, 2:3]
            rbl128 = sc128[:, 3:4]
            # (1-g)-scaled k/v weights (DVE; waits only on wk/wv + omg128)
            nc.vector.tensor_scalar_mul(wkb[:], wk_sb[:], omg128[:, 0:1])
            nc.vector.tensor_scalar_mul(wvb[:], wv_sb[:], omg128[:, 0:1])

            # PE fillers AFTER the pack matmuls: dependency-chained dummy MMs
            # keep the HAM clock warm through the initial x-DMA window.
            fillp = tc.alloc_tile_pool(name="fill_ps", bufs=1, space="PSUM")
            fps = fillp.tile([128, 128], dt.float32, tag="fill")
            for i in range(60):
                nc.tensor.matmul(fps[:], lhsT=ident[:], rhs=ident[:], start=(i == 0), stop=(i == 59))

            # scaled v_bias (1-g)*v_bias, transposed to [128,1] via PE
            # (emitted after the fillers: its input chain lands ~5us in)
            vbs = cp.tile([1, HD], dt.float32, tag="vbs")
            nc.vector.tensor_scalar_mul(vbs[:], vb[:], omg[:, 0:1])
            vbsT = cp.tile([128, 1], dt.float32, tag="vbsT")
            with tc.tile_pool(name="bc2_ps", bufs=1, space="PSUM") as bcp2:
                vb_ps = bcp2.tile([128, 1], dt.float32, tag="vb_ps")
                nc.tensor.matmul(vb_ps[:], lhsT=vbs[:], rhs=onef_row[:, 0:1], start=True, stop=True)
                nc.vector.tensor_copy(vbsT[:], vb_ps[:])

            # ---- persistent activations (raw proj, then normed in place) ----
            kT = cp.tile([128, t], dt.bfloat16, tag="kT")
            vT_sb = cp.tile([128, t], dt.bfloat16, tag="vT_sb")
            vnat = cp.tile([128, t], dt.bfloat16, tag="vnat")
            qT = cp.tile([128, HQ * t], dt.bfloat16, tag="qT")

            # ---- phase 1: group-major over 512-token columns ----
            # PE starts on q(g0) as soon as the first 0.5MB x group lands;
            # k/v trail one d-group behind their gpsimd blends. Per-column
            # epilogues (square/rowsum -> rsqrt chain -> broadcast+RoPE) are
            # deferred into the next column\'s MM stream.
            fillp.release()
            n_col = t // 512
            with (
                tc.tile_pool(name="xin", bufs=10) as xp,
                tc.tile_pool(name="skin", bufs=6) as skp,
                tc.tile_pool(name="stbp", bufs=20) as sbp,
                tc.tile_pool(name="p1s", bufs=2) as sp,
                tc.tile_pool(name="p1ps", bufs=1, space="PSUM") as psp,
                tc.tile_pool(name="p1row", bufs=2, space="PSUM") as rowp,
                tc.tile_pool(name="p1rb", bufs=1, space="PSUM") as rbp,
                tc.tile_pool(name="vtrp", bufs=1, space="PSUM") as vtrp,
            ):
                # rms_norm rsqrt via bit-trick ln + exp-table + 2 Newton steps
                # (keeps the ACT table on the exp set: no Ln -> no table swaps)
                LN2 = float(np.log(2.0))
                RS_SCALE = -0.5 * LN2 / (1 << 23)
                RS_BIAS = 0.5 * LN2 * (127 + 0.0450466) + 0.5 * float(np.log(HD))
                MINBITS = int(np.float32(1e-6).view(np.int32))
                rsb = cp.tile([128, 1], dt.float32, tag="rsb")
                nc.gpsimd.memset(rsb[:], RS_BIAS)

                deferred = []

                def pop_deferred(n=1):
                    for _ in range(n):
                        if deferred:
                            deferred.pop(0)()

                xg = {}
                sk = {}
                stb = {}

                def emit_col_dmas(c):
                    for g_ in range(n_grp):
                        xx = xp.tile([128, GD * 512], dt.bfloat16, tag="xg")
                        engx = nc.sync if (c + g_) % 2 == 0 else nc.scalar
                        engx.dma_start(
                            out=xx[:].rearrange("p (k f) -> p k f", k=GD),
                            in_=xT_d[128 * GD * g_ : 128 * GD * (g_ + 1), 512 * c : 512 * (c + 1)].rearrange("(k p) f -> p k f", p=128),
                        )
                        xg[(c, g_)] = xx
                        ss = skp.tile([128, GD * 512], dt.bfloat16, tag="sg")
                        engs = nc.scalar if (c + g_) % 2 == 0 else nc.sync
                        engs.dma_start(
                            out=ss[:].rearrange("p (k f) -> p k f", k=GD),
                            in_=skT_d[128 * GD * g_ : 128 * GD * (g_ + 1), 512 * c : 512 * (c + 1)].rearrange("(k p) f -> p k f", p=128),
                        )
                        sk[(c, g_)] = ss

                def emit_blends(c, g_):
                    # stb = x + (g/(1-g))*skip (DVE scalar_tensor_tensor;
                    # TensorScalarPtr is illegal on the Pool engine)
                    for kk in range(GD):
                        k_ = GD * g_ + kk
                        bl = sbp.tile([128, 512], dt.bfloat16, tag="stb")
                        nc.vector.scalar_tensor_tensor(
                            bl[:], sk[(c, g_)][:, 512 * kk : 512 * (kk + 1)], rbl128[:, 0:1],
                            xg[(c, g_)][:, 512 * kk : 512 * (kk + 1)], ALU.mult, ALU.add,
                        )
                        stb[(c, k_)] = bl

                def unit_mms(c, kind, g_, col_ps):
                    ps = col_ps[kind]
                    for j in range(GD):
                        k_ = GD * g_ + j
                        if kind == "k" or kind == "v":
                            w = wkb if kind == "k" else wvb
                            lhsT = w[:, k_ * HD : (k_ + 1) * HD]
                            rhs = stb[(c, k_)][:]
                        else:
                            h = int(kind[1])
                            lhsT = wq_sb[:, k_ * HQ * HD + h * HD : k_ * HQ * HD + (h + 1) * HD]
                            rhs = xg[(c, g_)][:, 512 * j : 512 * (j + 1)]
                        nc.tensor.matmul(ps[:], lhsT=lhsT, rhs=rhs, start=(k_ == 0), stop=(k_ == n_dt - 1))

                def col_sq_ssq(raw_cs, rowps, u):
                    sq = sp.tile([128, 512], dt.bfloat16, tag=f"sq{u}")
                    nc.scalar.square(sq[:], raw_cs)
                    nc.tensor.matmul(rowps[32 * u : 32 * u + 1, :], lhsT=ones_col[:], rhs=sq[:], start=True, stop=True)

                def rsq_chain(rowps):
                    """y2[32u,:] ~= rsqrt(rowps[32u,:]/HD), all rows at once.
                    Newton ladder ping-pongs ACT (square, 1.5-z affine) and
                    DVE (multiplies) so neither engine eats the whole cost."""
                    A = sp.tile([128, 512], dt.float32, tag="rsA")
                    B = sp.tile([128, 512], dt.float32, tag="rsB")
                    S = sp.tile([128, 512], dt.float32, tag="rsS")
                    nc.vector.tensor_scalar(S[:], rowps[:], 0.5 / HD, None, ALU.mult)
                    nc.vector.tensor_scalar(A[:], rowps[:].bitcast(dt.int32), MINBITS, None, ALU.max)
                    nc.scalar.activation(B[:], A[:], AF.Exp, bias=rsb[:, 0:1], scale=RS_SCALE)
                    nc.scalar.square(A[:], B[:])
                    nc.vector.tensor_mul(A[:], A[:], S[:])
                    nc.scalar.activation(A[:], A[:], AF.Identity, bias=c15s[:, 0:1], scale=-1.0)
                    nc.vector.tensor_mul(A[:], B[:], A[:])  # A = y1
                    nc.scalar.square(B[:], A[:])
                    nc.vector.tensor_mul(B[:], B[:], S[:])
                    nc.scalar.activation(B[:], B[:], AF.Identity, bias=c15s[:, 0:1], scale=-1.0)
                    nc.vector.tensor_mul(B[:], A[:], B[:])  # B = y2
                    return B

                def col_part2(raw_cs, c, y2, u):
                    """PE broadcast (per-head gain^2/sqrt(HD) folded into the
                    q lhsT rows of growq) + normalize + RoPE, one column."""
                    cs = slice(512 * c, 512 * (c + 1))
                    qh = sp.tile([128, 512], dt.bfloat16, tag="qh")
                    rb = rbp.tile([128, 512], dt.float32, tag="rb")
                    nc.tensor.matmul(rb[:], lhsT=growq[32 * u : 32 * u + 1, :], rhs=y2[32 * u : 32 * u + 1, :], start=True, stop=True)
                    nc.vector.tensor_mul(qh[:], raw_cs, rb[:])
                    qsw = sp.tile([128, 512], dt.bfloat16, tag="qsw")
                    nc.vector.tensor_copy(qsw[0:64, :], qh[64:128, :])
                    nc.vector.tensor_copy(qsw[64:128, :], qh[0:64, :])
                    tsw = sp.tile([128, 512], dt.bfloat16, tag="tsw")
                    nc.vector.tensor_mul(tsw[:], qsw[:], sinF[:, cs])
                    tco = sp.tile([128, 512], dt.bfloat16, tag="tco")
                    nc.vector.tensor_mul(tco[:], qh[:], cosF[:, cs])
                    nc.vector.tensor_add(raw_cs, tco[:], tsw[:])

                def vnat_transpose(c):
                    vtp = vtrp.tile([128, 512], dt.bfloat16, tag="vtr")
                    for s_ in range(4):
                        kt = 4 * c + s_
                        nc.tensor.transpose(vtp[:, 128 * s_ : 128 * (s_ + 1)], vT_sb[:, 128 * kt : 128 * (kt + 1)], ident[:])
                    nc.vector.tensor_copy(vnat[:, 512 * c : 512 * (c + 1)], vtp[:])

                emit_col_dmas(0)
                nc.sync.dma_start(out=cosF[:], in_=cosF_d[:])
                nc.scalar.dma_start(out=sinF[:], in_=sinF_d[:])
                for c in range(n_col):
                    if c + 1 < n_col:
                        emit_col_dmas(c + 1)
                    col_ps = {kd: psp.tile([128, 512], dt.float32, tag=f"ps_{kd}", name=f"ps_{kd}") for kd in ("q0", "q1", "k", "v")}
                    for g_ in range(n_grp):
                        pop_deferred(2)
                        unit_mms(c, "q0", g_, col_ps)
                        unit_mms(c, "q1", g_, col_ps)
                        emit_blends(c, g_)
                        if g_ > 0:
                            unit_mms(c, "k", g_ - 1, col_ps)
                            unit_mms(c, "v", g_ - 1, col_ps)
                    pop_deferred(1)
                    unit_mms(c, "k", n_grp - 1, col_ps)
                    unit_mms(c, "v", n_grp - 1, col_ps)
                    # drains (q/k on ACT which is idle here; v adds its bias)
                    cs = slice(512 * c, 512 * (c + 1))
                    q0_cs = qT[:, t * 0 + 512 * c : t * 0 + 512 * (c + 1)]
                    q1_cs = qT[:, t * 1 + 512 * c : t * 1 + 512 * (c + 1)]
                    k_cs = kT[:, cs]
                    nc.scalar.activation(q0_cs, col_ps["q0"][:], AF.Copy)
                    nc.scalar.activation(q1_cs, col_ps["q1"][:], AF.Copy)
                    nc.scalar.activation(k_cs, col_ps["k"][:], AF.Copy)
                    nc.scalar.activation(vT_sb[:, cs], col_ps["v"][:], AF.Identity, bias=vbsT[:, 0:1], scale=1.0)
                    # defer the normalization epilogue into the next column
                    rowps = rowp.tile([128, 512], dt.float32, tag="rowps")
                    y2h = {}
                    raws = {0: q0_cs, 1: q1_cs, 2: k_cs}

                    def mk_sq(u, raws=raws, rowps=rowps):
                        def f():
                            col_sq_ssq(raws[u], rowps, u)
                        return f

                    def mk_chain(raws=raws, rowps=rowps, y2h=y2h):
                        def f():
                            col_sq_ssq(raws[2], rowps, 2)
                            y2h[0] = rsq_chain(rowps)
                        return f

                    def mk_p2(u, c=c, raws=raws, y2h=y2h):
                        def f():
                            col_part2(raws[u], c, y2h[0], u)
                        return f

                    deferred.append(mk_sq(0))
                    deferred.append(mk_sq(1))
                    deferred.append(mk_chain())
                    deferred.append(mk_p2(0))
                    deferred.append(mk_p2(1))
                    deferred.append(mk_p2(2))
                    deferred.append(lambda c=c: vnat_transpose(c))

                # drain remaining deferred work
                for d in deferred:
                    d()
                deferred = []
            wraw.release()

            # ---- phase 2: attention ----
            y_in = [dp.tile([NCORES, HD, rows], dt.bfloat16, name=f"y_in{h}", tag=f"y_in{h}") for h in range(HQ)]
            y_out = [dp.tile([NCORES, HD, rows], dt.bfloat16, name=f"y_out{h}", tag=f"y_out{h}") for h in range(HQ)]

            # wproj prefetch (full, both HWDGE rings) during phase 2;
            # one 2 MB coalesced DMA per column block
            prp = tc.alloc_tile_pool(name="pr_s", bufs=4)
            wps = {}
            for n in range(D // 512):
                wp = prp.tile([128, n_dt * 512], dt.bfloat16, name=f"wp{n}", tag="wp")
                eng = nc.sync if n % 2 == 0 else nc.scalar
                eng.dma_start(
                    out=wp[:].rearrange("p (k f) -> p k f", k=n_dt),
                    in_=wpT_d[:, 512 * n : 512 * (n + 1)].rearrange("(k p) f -> p k f", p=128),
                )
                wps[n] = wp

            ytp_ = tc.alloc_tile_pool(name="yt_s", bufs=1)
            yt_blocks = [None] * n_dt
            with (
                tc.tile_pool(name="att_s", bufs=5) as ap_,
                tc.tile_pool(name="acc_s", bufs=2) as ap2,
                tc.tile_pool(name="st_ps", bufs=3, space="PSUM") as stp_,
                tc.tile_pool(name="yl_ps", bufs=2, space="PSUM") as ylp_,
            ):
                def epi_part1(h, c, ytp, acc):
                    # l = ones.T @ (acc halves); 1/l on the DVE (table-free)
                    lrow_t = stp_.tile([128, 1024], dt.float32, tag="st", name="lrow_t")
                    lrow = lrow_t[0:1, 0:512]
                    nc.tensor.matmul(lrow, lhsT=ones_col[:], rhs=acc[:, 0:512], start=True, stop=False)
                    nc.tensor.matmul(lrow, lhsT=ones_col[:], rhs=acc[:, 512:1024], start=False, stop=True)
                    rl = ap_.tile([1, 512], dt.float32, tag="rl")
                    nc.vector.reciprocal_approx_fast(out=rl[:], in_=lrow)
                    return (h, c, ytp, rl)

                def epi_part2(h, c, ytp, rl):
                    # broadcast 1/l, normalize, ship pieces
                    rb2_t = stp_.tile([128, 1024], dt.float32, tag="st", name="rb2_t")
                    rb2 = rb2_t[:, 0:512]
                    nc.tensor.matmul(rb2, lhsT=onef_row[:], rhs=rl[:], start=True, stop=True)
                    rb2s = ap_.tile([128, 512], dt.float32, tag="rb2s")
                    nc.vector.tensor_copy(rb2s[:], rb2)
                    ysb = ap_.tile([128, 512], dt.bfloat16, tag="ysb")
                    nc.vector.tensor_mul(ysb[:], ytp[:], rb2s[:])
                    for b in range(512 // rows):
                        piece = (512 * c) // rows + b
                        nc.sync.dma_start(
                            out=y_in[h][piece, :, :],
                            in_=ysb[:, rows * b : rows * (b + 1)],
                        )

                prev_epi = None  # (h, c, ytp, acc): chunk awaiting part1
                epi1 = None  # (h, c, ytp, rl): awaiting part2
                for h in range(HQ):
                    for c in range(n_ch):
                        qs = slice(t * h + 512 * c, t * h + 512 * (c + 1))
                        nkts = kpc * (c + 1)
                        ytp = ylp_.tile([128, 512], dt.float32, tag="yt")
                        # single 1024-wide denominator accumulator: one DVE op
                        # per 2-kt group instead of two 512-wide ones
                        acc = ap2.tile([128, 1024], dt.bfloat16, tag="acc")
                        pend = []  # [(pp, kts)] awaiting y/acc emission (2-deep)

                        def emit_pend(p, last, ytp=ytp, acc=acc):
                            ppp, kts_ = p
                            for s_, kt_ in enumerate(kts_):
                                pseg = ppp[:, 512 * s_ : 512 * (s_ + 1)]
                                nc.tensor.matmul(ytp[:], lhsT=vnat[:, HD * kt_ : HD * (kt_ + 1)], rhs=pseg, start=(kt_ == 0), stop=(last and kt_ == kts_[-1]))
                            if kts_[0] == 0:
                                nc.vector.tensor_copy(acc[:], ppp[:])
                            else:
                                nc.vector.tensor_add(acc[:], acc[:], ppp[:])

                        for pgi in range(nkts // 2):
                            kts = [2 * pgi, 2 * pgi + 1]
                            stp = stp_.tile([128, 1024], dt.float32, tag="st")
                            for s, kt in enumerate(kts):
                                seg = stp[:, 512 * s : 512 * (s + 1)]
                                diag = kt >= kpc * c
                                nc.tensor.matmul(seg, lhsT=kT[:, 128 * kt : 128 * (kt + 1)], rhs=qT[:, qs], start=True, stop=not diag)
                                if diag:
                                    m = kt - kpc * c
                                    nc.tensor.matmul(seg, lhsT=ident[:], rhs=mask[:, 512 * m : 512 * (m + 1)], start=False, stop=True)
                            # 2-deep lookahead: y/acc for group i-2 land after
                            # scores of group i, so exp latency is fully hidden
                            if len(pend) >= 2:
                                emit_pend(pend.pop(0), last=False)
                            pp = ap_.tile([128, 1024], dt.bfloat16, tag="pp")
                            nc.scalar.activation(pp[:], stp[:], AF.Exp, bias=negCmax[:, 0:1], scale=1.0)
                            npg = nkts // 2
                            p1_at = min(npg - 2, 3) if npg >= 3 else 0
                            if pgi == p1_at and prev_epi is not None:
                                epi1 = epi_part1(*prev_epi)
                                prev_epi = None
                            elif pgi == p1_at + 1 and epi1 is not None:
                                epi_part2(*epi1)
                                epi1 = None
                            pend.append((pp, kts))
                        while pend:
                            emit_pend(pend.pop(0), last=(len(pend) == 0))
                        prev_epi = (h, c, ytp, acc)
                    # flush last chunk's epilogue before the collective
                    epi_part2(*epi_part1(*prev_epi))
                    prev_epi = None
                    nc.gpsimd.collective_compute(
                        "AllToAll",
                        ALU.bypass,
                        replica_groups=[list(range(NCORES))],
                        ins=[y_in[h].opt()],
                        outs=[y_out[h].opt()],
                    )
                    if h == 0:
                        yb = ytp_.tile([128, NCORES * rows], dt.bfloat16, name="ytall0", tag="ytall0")
                        nc.sync.dma_start(
                            out=yb[:].rearrange("p (j r) -> p j r", j=NCORES),
                            in_=y_out[0][:].rearrange("j p r -> p j r"),
                        )
                        for j in range(NCORES):
                            yt_blocks[2 * j] = yb[:, rows * j : rows * (j + 1)]

            # ---- phase 3: output projection (h0 pass overlaps A2A(h1)) ----
            yb1 = ytp_.tile([128, NCORES * rows], dt.bfloat16, name="ytall1", tag="ytall1")
            nc.sync.dma_start(
                out=yb1[:].rearrange("p (j r) -> p j r", j=NCORES),
                in_=y_out[1][:].rearrange("j p r -> p j r"),
            )
            for j in range(NCORES):
                yt_blocks[2 * j + 1] = yb1[:, rows * j : rows * (j + 1)]

            mb = min(128, rows)
            nb = rows // mb
            tiles3 = [(n, b) for n in range(D // 512) for b in range(nb)]
            with (
                tc.tile_pool(name="pr_ps", bufs=1, space="PSUM") as prps,
                tc.tile_pool(name="pr_out", bufs=2) as prout,
            ):
                opss = {}
                for (n, b) in tiles3:
                    ops = prps.tile([mb, 512], dt.float32, tag=f"ops{n}_{b}")
                    opss[(n, b)] = ops
                    for ai, a in enumerate(range(0, n_dt, 2)):  # h0 blocks
                        nc.tensor.matmul(
                            ops[:],
                            lhsT=yt_blocks[a][:, mb * b : mb * (b + 1)],
                            rhs=wps[n][:, 512 * a : 512 * (a + 1)],
                            start=(ai == 0),
                            stop=False,
                        )
                for (n, b) in tiles3:
                    ops = opss[(n, b)]
                    for ai, a in enumerate(range(1, n_dt, 2)):  # h1 blocks
                        nc.tensor.matmul(
                            ops[:],
                            lhsT=yt_blocks[a][:, mb * b : mb * (b + 1)],
                            rhs=wps[n][:, 512 * a : 512 * (a + 1)],
                            start=False,
                            stop=(ai == n_dt // 2 - 1),
                        )
                    osb = prout.tile([mb, 512], dt.bfloat16, tag="osb")
                    nc.scalar.activation(osb[:], ops[:], AF.Copy, scale=lns128[:mb, 0:1])
                    nc.sync.dma_start(
                        out=out_d[mb * b : mb * (b + 1), 512 * n : 512 * (n + 1)],
                        in_=osb[:],
                    )
            ytp_.release()
            prp.release()
    nc.finalize()
    return nc


def make_tables(t=T):
    pos = np.arange(t, dtype=np.float32)
    inv = 1.0 / (ROPE_BASE ** (np.arange(0, HD, 2, dtype=np.float32) / HD))
    fr = pos[:, None] * inv[None, :]  # [t, 64]
    cos = np.cos(fr).T  # [64, t]
    sin = np.sin(fr).T
    cosF = np.concatenate([cos, cos], axis=0)  # [128, t]
    sinF = np.concatenate([sin, -sin], axis=0)
    return _bf(cosF), _bf(sinF)


def make_masks():
    # mask[p, 512*m + j] = 0 if j >= 128*m + p else MASK_VAL
    p = np.arange(128)[:, None]
    j = np.arange(512)[None, :]
    blocks = [np.where(j >= 128 * m + p, 0.0, MASK_VAL) for m in range(4)]
    return _bf(np.concatenate(blocks, axis=1))


_GRAPH_CACHE = {}
_LAST_IN_MAPS = None


def kernel(x, skip, wq, wk, wv, wproj, qk_g, ln_s, v_bias):
    t = x.shape[1]
    if t not in _GRAPH_CACHE:
        _GRAPH_CACHE[t] = build_graph(t)
    nc = _GRAPH_CACHE[t]

    xT = _bf(x.reshape(t, D).T)
    skT = _bf(skip.reshape(t, D).T)
    wpT = _bf(np.asarray(wproj, np.float32).T)
    cosF, sinF = make_tables(t)
    masks = make_masks()
    ident = _bf(np.eye(128, dtype=np.float32))

    in_maps = []
    for c in range(NCORES):
        kv = c // 2
        in_maps.append(
            {
                "xT": xT,
                "skipT": skT,
                "wqT": _bf(np.asarray(wq, np.float32)[HQ * HD * c : HQ * HD * (c + 1), :].T),
                "wkT": _bf(np.asarray(wk, np.float32)[HD * kv : HD * (kv + 1), :].T),
                "wvT": _bf(np.asarray(wv, np.float32)[HD * kv : HD * (kv + 1), :].T),
                "wprojT": wpT,
                "qkg": np.asarray(qk_g, np.float32)[HQ * c : HQ * (c + 1)].reshape(1, HQ),
                "lns": np.asarray(ln_s, np.float32).reshape(1, 1),
                "vbias": np.asarray(v_bias, np.float32)[kv].reshape(1, HD),
                "cosF": cosF,
                "sinF": sinF,
                "masks": masks,
                "ident": ident,
            }
        )
    global _LAST_IN_MAPS
    _LAST_IN_MAPS = in_maps
    res = run_bass_kernel_spmd(nc, in_maps, list(range(NCORES)))
    out = np.concatenate(
        [np.asarray(res.results[c]["out"], np.float32) for c in range(NCORES)], axis=0
    )
    return out.reshape(1, t, D).astype(np.float32)

